# revision 28
# baseline (speedup 1.0000x reference)
"""Multi-head self-attention (B=2, S=2048, D=768, H=12) on 8 trn2 NeuronCores.

Sharding: core c = 4*b + g handles batch b and head-group g (3 heads = 192 of
the 768 model dims). Weights are column-split (wq/wk/wv) and row-split (wo);
each core emits a partial (2048, 768) output; the host sums the 4 group
partials per batch and adds bo.

Device-side dataflow is transpose-free: inputs arrive pre-transposed (D, S),
so projections produce Q^T/K^T in (head_dim, S) layout which feeds the
logits matmul directly; softmax is computed as exp(logits/8) without
max-subtraction (logits are ~N(0,1), exp cannot overflow). The V tile for
each head carries 64 ones-columns after its 64 value columns, so the context
matmul emits the softmax denominator already broadcast across 64 psum
partitions (PE cycles depend only on the moving free size, so the extra
output rows are free); normalization is then one reciprocal_approx_fast and
one multiply on the vector engine, with no cross-partition DMA. The context
comes out transposed (dims, S), which is exactly the stationary operand the
output projection needs.

Matmul operands use bfloat16 (1 cycle/row on the PE at any free size, vs
float32r's 4 cycles/row below 256 free elements); accumulation stays fp32 in
PSUM.
"""
import numpy as np
from contextlib import ExitStack

import ml_dtypes

import concourse.bacc as bacc
import concourse.mybir as mybir
import concourse.tile as tile
from concourse import bass_utils

# Problem shape (hardcoded per contract).
B, S, D, H, DH = 2, 2048, 768, 12, 64
NCORES = 8
NG = 4            # head groups
HG = H // NG      # heads per group (3)
G = HG * DH       # model dims per group (192)
SC = 512          # query-chunk length
NQ = S // SC      # 4 chunks
KB = 128          # key-block length
NKB = S // KB     # 16 blocks
KT6 = D // 128    # 6 contraction tiles for the projections
SEG = 2 * DH      # V segment width per head: 64 V columns + 64 ones columns
FP32 = mybir.dt.float32
CDT = mybir.dt.bfloat16   # matmul-operand dtype
NPCDT = ml_dtypes.bfloat16

AF = mybir.ActivationFunctionType
ALU = mybir.AluOpType

_CACHE: dict = {}


def _build():
    nc = bacc.Bacc("TRN2", target_bir_lowering=False, debug=False)

    qT = nc.dram_tensor("qT", [NQ, 128, KT6, SC], CDT, kind="ExternalInput")
    kT = nc.dram_tensor("kT", [NQ, 128, KT6, SC], CDT, kind="ExternalInput")
    vT = nc.dram_tensor("vT", [NKB, 128, KT6, KB], CDT, kind="ExternalInput")
    wq = nc.dram_tensor("wq", [128, KT6, G], CDT, kind="ExternalInput")
    wk = nc.dram_tensor("wk", [128, KT6, G], CDT, kind="ExternalInput")
    wv = nc.dram_tensor("wv", [128, KT6, G], CDT, kind="ExternalInput")
    wo = nc.dram_tensor("wo", [G, D], CDT, kind="ExternalInput")
    bq = nc.dram_tensor("bq", [G, 1], FP32, kind="ExternalInput")
    bk = nc.dram_tensor("bk", [G, 1], FP32, kind="ExternalInput")
    yp = nc.dram_tensor("yp", [S, D], FP32, kind="ExternalOutput")

    with tile.TileContext(nc) as tc, ExitStack() as ctx:
        const = ctx.enter_context(tc.tile_pool(name="const", bufs=1))
        # 5 kx/qx buffers + 6 vx buffers: deep enough rings that a queued
        # input DMA never waits on a buffer still being read (an unmet WAR
        # blocks the in-order sync queue and every DMA behind it).
        xin = ctx.enter_context(tc.tile_pool(name="xin", bufs=5))
        vxp = ctx.enter_context(tc.tile_pool(name="vxp", bufs=4))
        qtp = ctx.enter_context(tc.tile_pool(name="qtp", bufs=2))
        ppool = ctx.enter_context(tc.tile_pool(name="ppool", bufs=2))
        ctxp = ctx.enter_context(tc.tile_pool(name="ctxp", bufs=2))
        ypool = ctx.enter_context(tc.tile_pool(name="ypool", bufs=2))
        den = ctx.enter_context(tc.tile_pool(name="den", bufs=2))
        ps_proj = ctx.enter_context(tc.tile_pool(name="ps_proj", bufs=2, space="PSUM"))
        ps_log = ctx.enter_context(tc.tile_pool(name="ps_log", bufs=2, space="PSUM"))
        ps_ctx = ctx.enter_context(tc.tile_pool(name="ps_ctx", bufs=2, space="PSUM"))

        # ---- constants / weights ------------------------------------------
        # DMA issue order = consumption order: the stream is DMA-bound for
        # its first ~25us, so everything is enqueued by deadline. No PE
        # warm-up: with bf16 transfer sizes the first projection inputs
        # arrive (~6us) before any engine-gated warm-up could even start
        # (compute engines issue their first op only ~6-7us after boot).
        # Each dma_start costs ~0.55us of in-order sync-engine dispatch, so
        # transfers are as few and as large as deadlines allow: kx0/qx0
        # split once (the first projection matmuls need only their first
        # k-tiles), vT grouped four blocks per DMA, biases after kx1.
        wk_sb = const.tile([128, KT6, G], CDT)
        kx0 = xin.tile([128, KT6, SC], CDT, tag="kx", name="kx_0")
        wq_sb = const.tile([128, KT6, G], CDT)
        qx0 = xin.tile([128, KT6, SC], CDT, tag="kx", name="qx_0")
        nc.sync.dma_start(wk_sb[:], wk.ap()[:, :, :])
        nc.sync.dma_start(kx0[:, 0:3, :], kT.ap()[0, :, 0:3, :])
        nc.sync.dma_start(kx0[:, 3:KT6, :], kT.ap()[0, :, 3:KT6, :])
        nc.sync.dma_start(wq_sb[:], wq.ap()[:, :, :])
        nc.sync.dma_start(qx0[:, 0:3, :], qT.ap()[0, :, 0:3, :])
        nc.sync.dma_start(qx0[:, 3:KT6, :], qT.ap()[0, :, 3:KT6, :])
        kxs = {}
        kxs[1] = xin.tile([128, KT6, SC], CDT, tag="kx", name="kx_1")
        nc.sync.dma_start(kxs[1][:], kT.ap()[1])
        bq0 = const.tile([128, 1], FP32)
        nc.sync.dma_start(bq0[:], bq.ap()[0:128, :])
        bq1 = const.tile([64, 1], FP32)
        nc.sync.dma_start(bq1[:], bq.ap()[128:G, :])
        bk0 = const.tile([128, 1], FP32)
        nc.sync.dma_start(bk0[:], bk.ap()[0:128, :])
        bk1 = const.tile([64, 1], FP32)
        nc.sync.dma_start(bk1[:], bk.ap()[128:G, :])
        for c in range(2, NQ):
            kxs[c] = xin.tile([128, KT6, SC], CDT, tag="kx", name=f"kx_{c}")
            nc.sync.dma_start(kxs[c][:], kT.ap()[c])
        wv_sb = const.tile([128, KT6, G], CDT)
        nc.sync.dma_start(wv_sb[:], wv.ap()[:, :, :])
        vblk = []
        for g4 in range(4):
            t = vxp.tile([128, 4, KT6, KB], CDT, tag="vx", name=f"vblk_{g4}")
            nc.sync.dma_start(
                t[:], vT.ap()[4 * g4:4 * (g4 + 1)].rearrange("b p t k -> p b t k")
            )
            vblk.append(t)
        wo_sb0 = const.tile([128, D], CDT)
        nc.sync.dma_start(wo_sb0[:], wo.ap()[0:128, :])
        wo_sb1 = const.tile([128, D], CDT)
        nc.sync.dma_start(wo_sb1[0:64, :], wo.ap()[128:G, :])

        # K^T per head, zero-padded to a full 128-partition contraction.
        # Partition placement matches the stacked Q^T tiles, so the padding
        # rows multiply zeros (or real rows multiply zero Q halves) and
        # every logits matmul runs with a full-height stationary — a
        # half-height (K=64) stationary makes the PE look half-idle to the
        # activity monitor, which then clamps the clock to half rate.
        KTz0 = const.tile([128, S], CDT)   # [K_h0^T ; 0]
        KTz1 = const.tile([128, S], CDT)   # [0 ; K_h1^T]
        KTz2 = const.tile([128, S], CDT)   # [K_h2^T ; 0]
        # KTz0 is needed first (first logits ~9us in), so it goes on the
        # vector engine; everything else initializes on the otherwise-idle
        # gpsimd engine to keep the vector queue clear for evictions.
        nc.vector.memset(KTz0[64:128, :], 0.0)
        nc.vector.memset(KTz1[0:64, :], 0.0)
        nc.vector.memset(KTz2[64:128, :], 0.0)
        nc.vector.memset(wo_sb1[64:128, :], 0.0)
        # V blocks: per head 64 value columns then 64 ones columns, so the
        # context matmul also emits the denominator on psum rows 64..127.
        Vg = const.tile([128, NKB, HG * SEG], CDT)
        for h in range(HG):
            nc.vector.memset(Vg[:, :, h * SEG + DH:(h + 1) * SEG], 1.0)

        mblocks = ((128, 0), (64, 128))  # (rows, row-offset) of the 192 dims

        # ---- K^T / V projections as emission units -----------------------
        # These are DMA-bound; instead of running them as serial phases
        # (PE half-idle, HAM re-throttles), they are spread as PE filler
        # into the first stream slots, hiding the loads under attention.
        def kt_units(c, kx=None):
            if kx is None:
                kx = xin.tile([128, KT6, SC], CDT, tag="kx", name=f"kx_{c}")
                nc.sync.dma_start(kx[:], kT.ap()[c])
            state = {}
            units = []

            def mk_mm(m, mp, mo, tpair):
                def emit():
                    if m not in state:
                        state[m] = ps_proj.tile(
                            [128, SC], FP32, tag="pp", name=f"ktps_{c}_{m}"
                        )
                    ps = state[m]
                    for t in tpair:
                        nc.tensor.matmul(
                            ps[:mp, :], wk_sb[:, t, mo:mo + mp], kx[:, t, :],
                            start=(t == 0), stop=(t == KT6 - 1),
                        )
                return emit

            def mk_evict(m, mp):
                def emit():
                    sl = slice(c * SC, (c + 1) * SC)
                    if m == 0:
                        nc.vector.tensor_scalar_add(
                            KTz0[0:64, sl], state[m][0:64, :], bk0[0:64, :]
                        )
                        nc.vector.tensor_scalar_add(
                            KTz1[64:128, sl], state[m][64:128, :],
                            bk0[64:128, :],
                        )
                    else:
                        nc.vector.tensor_scalar_add(
                            KTz2[0:64, sl], state[m][0:64, :], bk1[0:64, :]
                        )
                return emit

            for m, (mp, mo) in enumerate(mblocks):
                for tp in ((0, 1), (2, 3), (4, 5)):
                    units.append(mk_mm(m, mp, mo, tp))
                units.append(mk_evict(m, mp))
            return units

        def v_units(sb):
            vx = vblk[sb // 4][:, sb % 4]
            state = {}
            units = []

            def mk_mm(tpl, last):
                def emit():
                    if "ps" not in state:
                        state["ps"] = ps_proj.tile(
                            [128, G], FP32, tag="pp", name=f"vps_{sb}"
                        )
                    ps = state["ps"]
                    for t in tpl:
                        nc.tensor.matmul(
                            ps[:], vx[:, t, :], wv_sb[:, t, :],
                            start=(t == 0), stop=(last and t == KT6 - 1),
                        )
                return emit

            def mk_evict():
                def emit():
                    # all 3 heads' value columns in one strided copy
                    nc.vector.tensor_copy(
                        Vg[:, sb, :].rearrange(
                            "p (h c) -> p h c", c=SEG
                        )[:, :, 0:DH],
                        state["ps"][:, :].rearrange(
                            "p (h c) -> p h c", c=DH
                        ),
                    )
                return emit

            units.append(mk_mm((0, 1, 2), False))
            units.append(mk_mm((3, 4, 5), True))
            units.append(mk_evict())
            return units

        # ---- phase 3: software-pipelined head stream ---------------------
        # Heads form one flat stream across chunks. Each slot interleaves
        # head i's logits+exp with head i-1's context matmuls so PE and ACT
        # both stay fed (in-order engines execute in emission order). The
        # normalization chain of head i-1 is emitted at slot end; the output
        # projection of a finished chunk is emitted one slot later, after
        # its normalization latency has been hidden under a full slot.
        QT = {}     # qc -> (QT0, QT1)
        CT = {}     # qc -> (ctxT0, ctxT1)

        KTZ = (KTz0, KTz1, KTz2)

        def head_slices(qc, h):
            qt0, qt1 = QT[qc]
            return KTZ[h], qt0 if h < 2 else qt1

        def qt_units(qc, qx=None):
            # QT projection broken into emission units (PE filler). The qx
            # DMA and tile allocations happen now; matmuls are emitted as
            # the units are drained inside a kb2 loop.
            if qx is None:
                qx = xin.tile([128, KT6, SC], CDT, tag="kx", name=f"qx_{qc}")
                nc.sync.dma_start(qx[:], qT.ap()[qc])
            QT0 = qtp.tile([128, SC], CDT, tag="qt0", name=f"QT0_{qc}")
            QT1 = qtp.tile([128, SC], CDT, tag="qt1", name=f"QT1_{qc}")
            nc.vector.memset(QT1[64:128, :], 0.0)
            QT[qc] = (QT0, QT1)
            units = []
            state = {}

            def mk_mm(m, mp, mo, tpair):
                def emit():
                    if m not in state:
                        state[m] = ps_proj.tile(
                            [128, SC], FP32, tag="pp", name=f"qtps_{qc}_{m}"
                        )
                    ps = state[m]
                    for t in tpair:
                        nc.tensor.matmul(
                            ps[:mp, :], wq_sb[:, t, mo:mo + mp], qx[:, t, :],
                            start=(t == 0), stop=(t == KT6 - 1),
                        )
                return emit

            def mk_evict(m, mp):
                def emit():
                    dst = QT0 if m == 0 else QT1
                    bias = bq0 if m == 0 else bq1
                    nc.vector.tensor_scalar_add(
                        dst[0:mp, :], state[m][0:mp, :], bias[0:mp, :]
                    )
                return emit

            for m, (mp, mo) in enumerate(mblocks):
                for tp in ((0, 1), (2, 3), (4, 5)):
                    units.append(mk_mm(m, mp, mo, tp))
                units.append(mk_evict(m, mp))
            return units

        def emit_qt_proj(qc, qx=None):
            for u in qt_units(qc, qx):
                u()

        def emit_norm(qc, h, pc):
            # Normalization: psum rows 64..127 already hold the denominator
            # broadcast across 64 partitions (ones-columns of Vg), so this
            # is one wide approximate reciprocal and one multiply, all on
            # the vector engine — no cross-partition DMA in the chain.
            # reciprocal_approx_fast's BITWISE_NOT seed needs a plain SBUF
            # fp32 read with aligned partitions (direct PSUM input produced
            # garbage on hw), so stage the denominator through SBUF first.
            # (gpsimd cannot access PSUM, so the whole chain stays on DVE)
            cu = den.tile([64, SC], FP32, tag="cu")
            nc.vector.tensor_copy(cu[:], pc[64:128, :])
            rec = den.tile([64, SC], FP32, tag="rec")
            nc.vector.reciprocal_approx_fast(rec[:], cu[:])
            ctxT0, ctxT1 = CT[qc]
            cdst = ctxT0[64 * h:64 * h + 64, :] if h < 2 else ctxT1[0:64, :]
            nc.vector.tensor_tensor(cdst, pc[0:64, :], rec[:], ALU.mult)

        def y_units(qc, flush=False):
            # Output projection as emission units (PE filler), split so the
            # ctxT0 half (heads 0/1, normalized a full slot ago) can run
            # before the ctxT1 half (head 2, normalized at the end of the
            # previous slot): a_units accumulate ctxT0 @ wo0 into psum,
            # b_units add ctxT1 @ wo1, evict, and ride the chunk-half DMA.
            # In flush mode there are no logits left, so the idle ps_log /
            # ps_ctx banks hold extra open accumulators (5 A-units can run
            # before the first B) and the evictions go to the idle ACT
            # engine instead of the backlogged vector engine.
            ctxT0, ctxT1 = CT[qc]
            ytiles = {}
            ptiles = {}
            a_units = []
            b_units = []
            flush_pools = {2: (ps_log, "pl"), 3: (ps_log, "pl"),
                           4: (ps_ctx, "pc"), 7: (ps_log, "pl")}

            def mk_a(i, half, m, nh):
                def emit():
                    sb = half * 2 + m
                    pool, tg = (flush_pools.get(i, (ps_proj, "pp"))
                                if flush else (ps_proj, "pp"))
                    py = pool.tile(
                        [128, D // 2], FP32, tag=tg, name=f"yps_{qc}_{sb}_{nh}"
                    )
                    ptiles[(half, m, nh)] = py
                    nc.tensor.matmul(
                        py[:],
                        ctxT0[:, sb * 128:(sb + 1) * 128],
                        wo_sb0[:, nh * (D // 2):(nh + 1) * (D // 2)],
                        start=True, stop=False,
                    )
                return emit

            def mk_b(half, m, nh):
                def emit():
                    if half not in ytiles:
                        ytiles[half] = ypool.tile(
                            [128, 2, D], FP32, tag="Y", name=f"Yt_{qc}_{half}"
                        )
                    Yt = ytiles[half]
                    sb = half * 2 + m
                    py = ptiles[(half, m, nh)]
                    nc.tensor.matmul(
                        py[:],
                        ctxT1[:, sb * 128:(sb + 1) * 128],
                        wo_sb1[:, nh * (D // 2):(nh + 1) * (D // 2)],
                        start=False, stop=True,
                    )
                    ydst = Yt[:, m, nh * (D // 2):(nh + 1) * (D // 2)]
                    if flush:
                        nc.scalar.activation(ydst, py[:], AF.Copy)
                    else:
                        nc.vector.tensor_copy(ydst, py[:])
                    if nh == 1:
                        # per-128-row output DMA: smaller final transfer
                        # shortens the kernel tail after the last matmul
                        r0 = qc * SC + half * 256 + m * 128
                        nc.sync.dma_start(
                            yp.ap()[r0:r0 + 128, :], Yt[:, m, :]
                        )
                return emit

            i = 0
            for half in range(2):
                for m in range(2):
                    for nh in range(2):
                        a_units.append(mk_a(i, half, m, nh))
                        b_units.append(mk_b(half, m, nh))
                        i += 1
            return a_units, b_units

        def y_filler(qc):
            # Interleave A/B so at most two output psum tiles are live:
            # A0 A1 B0 A2 B1 ... A7 B6 B7.
            a_units, b_units = y_units(qc)
            seq = [a_units[0]]
            for i in range(1, 8):
                seq.append(a_units[i])
                seq.append(b_units[i - 1])
            seq.append(b_units[7])
            return seq

        def emit_ctx_pair(prev, kb2):
            qc_p, h_p, P_p, pc_p = prev
            for j in range(2):
                kb = 2 * kb2 + j
                nc.tensor.matmul(
                    pc_p[:, :],
                    Vg[:, kb, h_p * SEG:(h_p + 1) * SEG],
                    P_p[:, kb, :],
                    start=(kb == 0), stop=(kb == NKB - 1),
                )

        stream = [(qc, h) for qc in range(NQ) for h in range(HG)]
        prev = None      # (qc, h, P, pc) of the head whose ctx is in flight

        # KT chunk 0 and QT(0) must fully precede the first logits, so they
        # are emitted as blocks; everything else streams in as filler.
        for u in kt_units(0, kx0):
            u()
        emit_qt_proj(0, qx0)

        for qc, h in stream:
            if h == 0:
                ctxT0_n = ctxp.tile([128, SC], CDT, tag="c0",
                                    name=f"ctxT0_{qc}")
                ctxT1_n = ctxp.tile([128, SC], CDT, tag="c1",
                                    name=f"ctxT1_{qc}")
                nc.vector.memset(ctxT1_n[64:128, :], 0.0)
                CT[qc] = (ctxT0_n, ctxT1_n)
            # PE filler for this slot: remaining K^T/V projection units in
            # the first two slots; later, Y of the chunk finished last slot
            # (h==1: its normalization has had a full slot to land) or the
            # next chunk's QT projection prefetch (h==2).
            filler = []
            start_iter = 0
            if (qc, h) == (0, 0):
                for c in range(1, NQ):
                    filler.extend(kt_units(c, kxs[c]))
                for sb in range(4):
                    filler.extend(v_units(sb))
            elif (qc, h) == (0, 1):
                for sb in range(4, NKB):
                    filler.extend(v_units(sb))
            elif h == 1 and qc >= 1:
                filler = y_filler(qc - 1)
                start_iter = 3
            elif h == HG - 1 and qc + 1 < NQ:
                filler = qt_units(qc + 1)

            kt_t, qt_t = head_slices(qc, h)
            P = ppool.tile([128, NKB, SC], CDT, tag="P")
            NIT = NKB // 2
            for kb2 in range(NIT):
                pl = ps_log.tile([128, 2, SC], FP32, tag="pl")
                for j in range(2):
                    kb = 2 * kb2 + j
                    nc.tensor.matmul(
                        pl[:, j, :],
                        kt_t[:, kb * KB:(kb + 1) * KB],
                        qt_t[:, :],
                        start=True, stop=True,
                    )
                nc.scalar.activation(
                    P[:, 2 * kb2:2 * kb2 + 2, :], pl[:],
                    AF.Exp, scale=1.0 / np.sqrt(DH)
                )
                if filler and kb2 >= start_iter:
                    n = -(-len(filler) // (NIT - kb2))
                    for _ in range(n):
                        filler.pop(0)()
                if prev is not None:
                    emit_ctx_pair(prev, kb2)
            for u in filler:
                u()
            if prev is not None:
                emit_norm(prev[0], prev[1], prev[3])
            pc = ps_ctx.tile([128, SC], FP32, tag="pc")
            prev = (qc, h, P, pc)

        # flush: context of the final head interleaved with the final
        # chunk's ctxT0 output-projection half (its heads 0/1 normalized
        # during the last stream slot), then the final norm, then the
        # ctxT1 half + output DMA.
        a_units, b_units = y_units(NQ - 1, flush=True)
        for kb2 in range(NKB // 2):
            emit_ctx_pair(prev, kb2)
        emit_norm(prev[0], prev[1], prev[3])
        # 5 A-units run on the PE while the norm chain computes on DVE;
        # B interleaves with the remaining A's such that every psum-ring
        # WAR points at an already-emitted B (reader) of the same buffer.
        for u in (a_units[0:5]
                  + [b_units[0], a_units[5], b_units[1], a_units[6],
                     b_units[2], a_units[7]]
                  + b_units[3:8]):
            u()

    nc.compile()
    return nc


def _get_nc():
    if "nc" not in _CACHE:
        _CACHE["nc"] = _build()
    return _CACHE["nc"]


def _tile_x(xb, chunk):
    # x (S, D) -> x^T tiled (S/chunk, 128, KT6, chunk), contiguous bf16
    xt = np.asarray(xb, dtype=np.float32).T
    return np.ascontiguousarray(
        xt.reshape(KT6, 128, S // chunk, chunk).transpose(2, 1, 0, 3)
    ).astype(NPCDT)


def _tile_w(w):
    # (D, G) -> (128, KT6, G) contiguous bf16
    w = np.asarray(w, dtype=np.float32)
    return np.ascontiguousarray(
        w.reshape(KT6, 128, G).transpose(1, 0, 2)
    ).astype(NPCDT)


def _in_maps(v, k, q, wq, bq, wk, bk, wv, bv, wo, bo):
    f32 = lambda a: np.ascontiguousarray(np.asarray(a, dtype=np.float32))
    bf16 = lambda a: np.ascontiguousarray(np.asarray(a, dtype=np.float32)).astype(NPCDT)
    qTb = [_tile_x(q[b], SC) for b in range(B)]
    kTb = [_tile_x(k[b], SC) for b in range(B)]
    vTb = [_tile_x(v[b], KB) for b in range(B)]
    maps = []
    for c in range(NCORES):
        b, g = divmod(c, NG)
        cols = slice(g * G, (g + 1) * G)
        maps.append({
            "qT": qTb[b],
            "kT": kTb[b],
            "vT": vTb[b],
            "wq": _tile_w(np.asarray(wq)[:, cols]),
            "wk": _tile_w(np.asarray(wk)[:, cols]),
            "wv": _tile_w(np.asarray(wv)[:, cols]),
            "wo": bf16(wo[cols, :]),
            "bq": f32(np.asarray(bq)[cols].reshape(G, 1)),
            "bk": f32(np.asarray(bk)[cols].reshape(G, 1)),
        })
    return maps


def kernel(v, k, q, wq, bq, wk, bk, wv, bv, wo, bo, _trace=False):
    nc = _get_nc()
    in_maps = _in_maps(v, k, q, wq, bq, wk, bk, wv, bv, wo, bo)
    res = bass_utils.run_bass_kernel_spmd(
        nc, in_maps, core_ids=list(range(NCORES)), trace=_trace
    )
    # softmax weights sum to 1, so the V bias shifts ctx by exactly bv;
    # its contribution to the output is the constant row bv @ wo + bo.
    corr = (np.asarray(bv, dtype=np.float64) @ np.asarray(wo, dtype=np.float64)
            + np.asarray(bo, dtype=np.float64)).astype(np.float32)
    out = np.empty((B, S, D), dtype=np.float32)
    for b in range(B):
        acc = res.results[4 * b]["yp"].astype(np.float32)
        for g in range(1, NG):
            acc = acc + res.results[4 * b + g]["yp"]
        out[b] = acc + corr[None, :]
    if _trace:
        kernel.last_result = res
    return out
